# revision 1
# baseline (speedup 1.0000x reference)
"""GQA causal attention kernel for 8 Trainium2 NeuronCores.

Sharding: data-parallel over batch (2) x tensor-parallel over head groups (4).
Core c handles batch b = c // 4 and head group g = c % 4 (query heads
4g..4g+3, KV head g, Wo rows 512g..512(g+1)).  Each core computes a full
[N, DIM] partial of the output projection; the host sums the 4 partials
per batch.

Matmuls run in bf16 (fp32r measured ~2 cycles/row on HW, bf16 1): inputs are
converted on the host; all PSUM accumulation stays fp32.

Per-core pipeline:
  1. QKV projections from host-pretransposed x^T; all 16 D-chunks resident in
     SBUF (bf16), full-depth PSUM accumulation, 3 waves of 8 PSUM banks.
  2. V^T -> V via PE transposes.
  3. Per (head, 512-wide q band): scores computed TRANSPOSED (S^T[k, q]) so
     softmax needs no P transposes; exp on ScalarE; softmax denominators
     accumulated in broadcast form via an all-ones [128,128] stationary
     matmul (one PSUM bank holds 128 identical rows of the row sums, so
     normalization is reciprocal + one multiply, all 128-partition DVE ops);
     O^T accumulated over k chunks in PSUM.
  4. Output projection consumes O^T directly as the stationary operand.
"""

import os
import numpy as np

B, N, DIM = 2, 2048, 2048
H, KVH, HD = 16, 4, 128
HQ = H // KVH          # query heads per core
SCALE = float(HD) ** -0.5
NT = N // 128          # 16 seq tiles
DC = DIM // 128        # 16 contraction chunks
NB = 4                 # q bands of 512
BW = N // NB           # 512 band width

_cache = {}


def _build():
    import concourse.bass as bass
    import concourse.bacc as bacc
    import concourse.tile as tile
    import concourse.mybir as mybir

    f32 = mybir.dt.float32
    bf16 = mybir.dt.bfloat16
    EXP = mybir.ActivationFunctionType.Exp

    nc = bacc.Bacc("TRN2", target_bir_lowering=False, debug=False)

    xT = nc.dram_tensor("xT", [DIM, N], bf16, kind="ExternalInput")
    wq = nc.dram_tensor("wq", [DIM, HQ * HD], bf16, kind="ExternalInput")
    wk = nc.dram_tensor("wk", [DIM, HD], bf16, kind="ExternalInput")
    wv = nc.dram_tensor("wv", [DIM, HD], bf16, kind="ExternalInput")
    wo = nc.dram_tensor("wo", [HQ * HD, DIM], bf16, kind="ExternalInput")
    m01 = nc.dram_tensor("m01", [128, 128], bf16, kind="ExternalInput")
    ident = nc.dram_tensor("ident", [128, 128], bf16, kind="ExternalInput")
    onesd = nc.dram_tensor("onesd", [128, 128], bf16, kind="ExternalInput")
    out = nc.dram_tensor("out", [N, DIM], f32, kind="ExternalOutput")

    with tile.TileContext(nc) as tc:
        from contextlib import ExitStack

        with ExitStack() as ctx:
            resident = ctx.enter_context(tc.tile_pool(name="resident", bufs=1))

            # --- resident tiles ---
            qt = resident.tile([128, HQ * N], bf16)        # Q^T all heads
            kt = resident.tile([128, N], bf16)             # K^T
            vnat = resident.tile([128, N], bf16)           # V (seq-major chunks)
            m01_sb = resident.tile([128, 128], bf16)
            id_sb = resident.tile([128, 128], bf16)
            ones_sb = resident.tile([128, 128], bf16)
            wo_sb = [resident.tile([128, DIM], bf16, tag=f"wo{h}", name=f"wo{h}")
                     for h in range(HQ)]
            ot_sb2 = [resident.tile([128, HQ * BW], bf16, tag=f"ot_sb{i}",
                                    name=f"ot_sb{i}") for i in range(2)]

            # ---------------- Phase 1: projections ----------------
            with nc.named_scope("proj"):
                with (
                    tc.tile_pool(name="xth", bufs=1) as xth_pool,
                    tc.tile_pool(name="wqh", bufs=1) as wqh_pool,
                    tc.tile_pool(name="wkv", bufs=1) as wkv_pool,
                    tc.tile_pool(name="pp", bufs=8, space="PSUM") as pp,
                    tc.tile_pool(name="vt", bufs=1) as vt_pool,
                ):
                    # PE warmup while input DMAs land: ~4.5us of matmul
                    # activity flips the HAM clock gate to 8/8 before real
                    # work starts. Reads uninitialized SBUF (values unused);
                    # sink DMA keeps the chain from being dead-code-eliminated.
                    warm = pp.tile([128, 512], f32, tag="acc")
                    for _ in range(20):
                        nc.tensor.matmul(warm[:], ones_sb[:], kt[:, 0:512])
                    wsink = vt_pool.tile([128, 512], f32, tag="wsink")
                    nc.vector.tensor_copy(wsink[:], warm[:])
                    sink_dram = nc.dram_tensor("warm_sink", [128, 512], f32,
                                               kind="Internal")
                    nc.sync.dma_start(sink_dram.ap(), wsink[:])
                    wk_sb = wkv_pool.tile([128, DC * HD], bf16, tag="wk")
                    wv_sb = wkv_pool.tile([128, DC * HD], bf16, tag="wv")
                    vtmp = vt_pool.tile([128, N], bf16)    # V^T before transpose

                    # x^T / Wq chunks first (first matmuls need them); each
                    # x^T chunk split in two so transfers spread across DMA
                    # queues and matmuls start on the first half.
                    xth = []
                    wqh = []
                    for d in range(DC):
                        wq_t = wqh_pool.tile([128, HQ * HD], bf16, tag=f"w{d}",
                                             name=f"wqh{d}")
                        nc.sync.dma_start(
                            wq_t[:], wq.ap()[d * 128:(d + 1) * 128, :])
                        wqh.append(wq_t)
                        xt_t = xth_pool.tile([128, N], bf16, tag=f"x{d}",
                                             name=f"xth{d}")
                        for hh in range(2):
                            nc.sync.dma_start(
                                xt_t[:, hh * 1024:(hh + 1) * 1024],
                                xT.ap()[d * 128:(d + 1) * 128,
                                        hh * 1024:(hh + 1) * 1024])
                        xth.append(xt_t)

                    # K/V weights as single strided DMAs (needed at wave 2)
                    nc.sync.dma_start(
                        wk_sb[:].rearrange("p (d c) -> p d c", d=DC),
                        wk.ap().rearrange("(d p) c -> p d c", p=128))
                    nc.sync.dma_start(
                        wv_sb[:].rearrange("p (d c) -> p d c", d=DC),
                        wv.ap().rearrange("(d p) c -> p d c", p=128))
                    nc.sync.dma_start(id_sb[:], ident.ap())
                    nc.sync.dma_start(ones_sb[:], onesd.ap())
                    nc.sync.dma_start(m01_sb[:], m01.ap())

                    # waves 0/1: Q^T for head pairs; wave 2: K^T + V^T
                    for wave in range(3):
                        for s in range(8):
                            acc = pp.tile([128, 512], f32, tag="acc")
                            if wave < 2:
                                h = wave * 2 + s // 4
                                t = s % 4
                                for d in range(DC):
                                    nc.tensor.matmul(
                                        acc[:],
                                        wqh[d][:, h * HD:(h + 1) * HD],
                                        xth[d][:, t * 512:(t + 1) * 512],
                                        start=(d == 0), stop=(d == DC - 1))
                                dst = qt[:, h * N + t * 512: h * N + (t + 1) * 512]
                            else:
                                w_sb = wk_sb if s < 4 else wv_sb
                                t = s % 4
                                for d in range(DC):
                                    nc.tensor.matmul(
                                        acc[:],
                                        w_sb[:, d * HD:(d + 1) * HD],
                                        xth[d][:, t * 512:(t + 1) * 512],
                                        start=(d == 0), stop=(d == DC - 1))
                                src_t = kt if s < 4 else vtmp
                                dst = src_t[:, t * 512:(t + 1) * 512]
                            nc.vector.tensor_copy(dst, acc[:])

                    # V^T -> V natural via PE transpose
                    for j in range(NT):
                        tp = pp.tile([128, 128], bf16, tag="acc")
                        nc.tensor.transpose(
                            tp[:], vtmp[:, j * 128:(j + 1) * 128], id_sb[:])
                        nc.vector.tensor_copy(vnat[:, j * 128:(j + 1) * 128], tp[:])

            # wo loads (needed from first outproj; emitted after proj DMAs)
            for h in range(HQ):
                nc.sync.dma_start(wo_sb[h][:], wo.ap()[h * 128:(h + 1) * 128, :])

            # ---------------- Phase 2: attention + out-projection ----------------
            with nc.named_scope("attn"):
                with (
                    tc.tile_pool(name="pt", bufs=4) as pt_pool,
                    tc.tile_pool(name="rr", bufs=2) as rr_pool,
                    tc.tile_pool(name="stage", bufs=4) as stage_pool,
                    tc.tile_pool(name="st", bufs=2, space="PSUM") as st_pool,
                    tc.tile_pool(name="sums", bufs=2, space="PSUM") as sums_pool,
                    tc.tile_pool(name="ot", bufs=2, space="PSUM") as ot_pool,
                ):
                    for I in range(NB):
                        jmax = 4 * I + 3
                        ot_sb = ot_sb2[I % 2]
                        for h in range(HQ):
                            otp = ot_pool.tile([128, BW], f32, tag="ot")
                            smp = sums_pool.tile([128, BW], f32, tag="sums")
                            for p in range((jmax + 1) // 2):
                                stp = st_pool.tile([128, 2 * BW], f32, tag="st")
                                for u in range(2):
                                    j = 2 * p + u
                                    o = j - 4 * I
                                    qlo = max(0, o) * 128
                                    nc.tensor.matmul(
                                        stp[:, u * BW + qlo:(u + 1) * BW],
                                        kt[:, j * 128:(j + 1) * 128],
                                        qt[:, h * N + I * BW + qlo:
                                           h * N + (I + 1) * BW])
                                ptp = pt_pool.tile([128, 2 * BW], bf16, tag="pt")
                                nc.scalar.activation(ptp[:], stp[:], EXP, scale=SCALE)
                                for u in range(2):
                                    j = 2 * p + u
                                    o = j - 4 * I
                                    qlo = max(0, o) * 128
                                    if o >= 0:
                                        # triangular boundary within first 128
                                        # cols of the processed range
                                        nc.vector.tensor_mul(
                                            ptp[:, u * BW + qlo: u * BW + qlo + 128],
                                            ptp[:, u * BW + qlo: u * BW + qlo + 128],
                                            m01_sb[:, 0:128])
                                    pslice = ptp[:, u * BW + qlo:(u + 1) * BW]
                                    # row sums in broadcast form (all-ones lhsT)
                                    nc.tensor.matmul(
                                        smp[:, qlo:], ones_sb[:], pslice,
                                        start=(j == 0), stop=(j == jmax))
                                    nc.tensor.matmul(
                                        otp[:, qlo:], vnat[:, j * 128:(j + 1) * 128],
                                        pslice,
                                        start=(j == 0), stop=(j == jmax))
                            # normalize: O^T * (1/sums), all [128, BW] DVE ops
                            rb_sb = rr_pool.tile([128, BW], f32, tag="rb")
                            nc.vector.reciprocal_approx_fast(rb_sb[:], smp[:])
                            nc.vector.tensor_mul(
                                ot_sb[:, h * BW:(h + 1) * BW], otp[:], rb_sb[:])
                        # out projection for this band
                        for t in range(4):
                            stg = stage_pool.tile([128, DIM], f32, tag="stg")
                            for dt in range(4):
                                opp = ot_pool.tile([128, 512], f32, tag="ot")
                                for h in range(HQ):
                                    nc.tensor.matmul(
                                        opp[:],
                                        ot_sb[:, h * BW + t * 128: h * BW + (t + 1) * 128],
                                        wo_sb[h][:, dt * 512:(dt + 1) * 512],
                                        start=(h == 0), stop=(h == HQ - 1))
                                nc.vector.tensor_copy(
                                    stg[:, dt * 512:(dt + 1) * 512], opp[:])
                            nc.sync.dma_start(
                                out.ap()[I * BW + t * 128: I * BW + (t + 1) * 128, :],
                                stg[:])

    nc.compile()
    return nc


def _get_nc():
    if "nc" not in _cache:
        _cache["nc"] = _build()
    return _cache["nc"]


def _host_inputs(x, Wq, Wk, Wv, Wo):
    import ml_dtypes
    bf = ml_dtypes.bfloat16
    x = np.asarray(x, dtype=np.float32)
    Wq = np.asarray(Wq, dtype=bf)
    Wk = np.asarray(Wk, dtype=bf)
    Wv = np.asarray(Wv, dtype=bf)
    Wo = np.asarray(Wo, dtype=bf)

    kk = np.arange(128)[:, None]
    qq = np.arange(128)[None, :]
    m01 = (qq >= kk).astype(bf)
    ident = np.eye(128, dtype=bf)
    onesd = np.ones((128, 128), dtype=bf)

    xTb = [np.ascontiguousarray(x[b].T).astype(bf) for b in range(B)]
    in_maps = []
    for c in range(8):
        b, g = c // 4, c % 4
        in_maps.append({
            "xT": xTb[b],
            "wq": np.ascontiguousarray(Wq[:, g * 512:(g + 1) * 512]),
            "wk": np.ascontiguousarray(Wk[:, g * HD:(g + 1) * HD]),
            "wv": np.ascontiguousarray(Wv[:, g * HD:(g + 1) * HD]),
            "wo": np.ascontiguousarray(Wo[g * 512:(g + 1) * 512, :]),
            "m01": m01,
            "ident": ident,
            "onesd": onesd,
        })
    return in_maps


def run(x, mask, Wq, Wk, Wv, Wo, trace=False, trace_cores=None):
    from concourse.bass_utils import run_bass_kernel_spmd

    nc = _get_nc()
    in_maps = _host_inputs(x, Wq, Wk, Wv, Wo)
    res = run_bass_kernel_spmd(
        nc, in_maps, core_ids=list(range(8)), trace=trace,
        trace_cores=trace_cores)
    full = np.empty((B, N, DIM), dtype=np.float32)
    for b in range(B):
        acc = res.results[b * 4 + 0]["out"].astype(np.float32).copy()
        for g in range(1, 4):
            acc += res.results[b * 4 + g]["out"]
        full[b] = acc
    return full, res


def kernel(x, mask, Wq, Wk, Wv, Wo):
    out, _ = run(x, mask, Wq, Wk, Wv, Wo, trace=False)
    return out



# revision 6
# speedup vs baseline: 1.0566x; 1.0566x over previous
"""GQA causal attention kernel for 8 Trainium2 NeuronCores.

Sharding: data-parallel over batch (2) x tensor-parallel over head groups (4).
Core c handles batch b = c // 4 and head group g = c % 4 (query heads
4g..4g+3, KV head g, Wo rows 512g..512(g+1)).  Each core computes a full
[N, DIM] partial of the output projection (bf16); the host sums the 4
partials per batch in fp32.

Matmuls run in bf16 (fp32r measured ~2 cycles/row on HW, bf16 1): inputs are
converted on the host; all PSUM accumulation stays fp32.

Per-core pipeline:
  1. QKV projections from host-pretransposed x^T, d-outer accumulation order
     (8 open PSUM banks per wave) so the PE consumes x^T/W chunks in DMA
     arrival order instead of head-of-line blocking on the full depth.
  2. V^T -> V via PE transposes.
  3. Attention is software-pipelined at 128-row k-block granularity: the
     scores matmul for block j+3 issues on the PE while exp(j) runs on the
     Scalar engine, so the PE never waits for the softmax chain.  Scores are
     computed TRANSPOSED (S^T[k, q]); softmax denominators accumulate in
     broadcast form via an all-ones stationary matmul; O^T accumulates over
     k blocks in PSUM.
  4. Output projection consumes O^T directly as the stationary operand; its
     emission is delayed two blocks into the next band so the PE has work
     while the last head's normalization (DVE) completes.
"""

import numpy as np

B, N, DIM = 2, 2048, 2048
H, KVH, HD = 16, 4, 128
HQ = H // KVH          # query heads per core
SCALE = float(HD) ** -0.5
NT = N // 128          # 16 seq tiles
DC = DIM // 128        # 16 contraction chunks
NB = 4                 # q bands of 512
BW = N // NB           # 512 band width
LA = 3                 # attention block lookahead (PE pipeline depth)

_cache = {}


def _build():
    import concourse.bass as bass
    import concourse.bacc as bacc
    import concourse.tile as tile
    import concourse.mybir as mybir

    f32 = mybir.dt.float32
    bf16 = mybir.dt.bfloat16
    EXP = mybir.ActivationFunctionType.Exp

    nc = bacc.Bacc("TRN2", target_bir_lowering=False, debug=False)

    xT = nc.dram_tensor("xT", [DIM, N], bf16, kind="ExternalInput")
    wq = nc.dram_tensor("wq", [DIM, HQ * HD], bf16, kind="ExternalInput")
    wk = nc.dram_tensor("wk", [DIM, HD], bf16, kind="ExternalInput")
    wv = nc.dram_tensor("wv", [DIM, HD], bf16, kind="ExternalInput")
    wo = nc.dram_tensor("wo", [HQ * HD, DIM], bf16, kind="ExternalInput")
    m01 = nc.dram_tensor("m01", [128, 128], bf16, kind="ExternalInput")
    ident = nc.dram_tensor("ident", [128, 128], bf16, kind="ExternalInput")
    onesd = nc.dram_tensor("onesd", [128, 128], bf16, kind="ExternalInput")
    out = nc.dram_tensor("out", [N, DIM], bf16, kind="ExternalOutput")

    with tile.TileContext(nc) as tc:
        from contextlib import ExitStack

        with ExitStack() as ctx:
            resident = ctx.enter_context(tc.tile_pool(name="resident", bufs=1))

            # --- resident tiles ---
            qt = resident.tile([128, HQ * N], bf16)        # Q^T all heads
            kt = resident.tile([128, N], bf16)             # K^T
            vnat = resident.tile([128, N], bf16)           # V (seq-major chunks)
            m01_sb = resident.tile([128, 128], bf16)
            id_sb = resident.tile([128, 128], bf16)
            ones_sb = resident.tile([128, 128], bf16)
            wo_sb = [resident.tile([128, DIM], bf16, tag=f"wo{h}", name=f"wo{h}")
                     for h in range(HQ)]
            ot_sb2 = [resident.tile([128, HQ * BW], bf16, tag=f"ot_sb{i}",
                                    name=f"ot_sb{i}") for i in range(2)]

            # ---------------- Phase 1: projections ----------------
            with nc.named_scope("proj"):
                with (
                    tc.tile_pool(name="xth", bufs=1) as xth_pool,
                    tc.tile_pool(name="wqh", bufs=1) as wqh_pool,
                    tc.tile_pool(name="wkv", bufs=1) as wkv_pool,
                    tc.tile_pool(name="pp", bufs=8, space="PSUM") as pp,
                    tc.tile_pool(name="vt", bufs=1) as vt_pool,
                ):
                    # PE warmup while input DMAs land: matmul activity flips
                    # the HAM clock gate to 8/8 before real work starts.
                    # Reads uninitialized SBUF (values unused); sink DMA keeps
                    # the chain from being dead-code-eliminated.
                    warm = pp.tile([128, 512], f32, tag="acc")
                    for _ in range(12):
                        nc.tensor.matmul(warm[:], ones_sb[:], kt[:, 0:512])
                    wsink = vt_pool.tile([128, 512], f32, tag="wsink")
                    nc.vector.tensor_copy(wsink[:], warm[:])
                    sink_dram = nc.dram_tensor("warm_sink", [128, 512], f32,
                                               kind="Internal")
                    nc.sync.dma_start(sink_dram.ap(), wsink[:])
                    wk_sb = wkv_pool.tile([128, DC * HD], bf16, tag="wk")
                    wv_sb = wkv_pool.tile([128, DC * HD], bf16, tag="wv")
                    vtmp = vt_pool.tile([128, N], bf16)    # V^T before transpose

                    # x^T / Wq chunks first (first matmuls need them); each
                    # x^T chunk split in two so transfers spread across DMA
                    # queues and matmuls start on the first half.
                    xth = []
                    wqh = []
                    for d in range(DC):
                        wq_t = wqh_pool.tile([128, HQ * HD], bf16, tag=f"w{d}",
                                             name=f"wqh{d}")
                        nc.sync.dma_start(
                            wq_t[:], wq.ap()[d * 128:(d + 1) * 128, :])
                        wqh.append(wq_t)
                        xt_t = xth_pool.tile([128, N], bf16, tag=f"x{d}",
                                             name=f"xth{d}")
                        for hh in range(2):
                            nc.sync.dma_start(
                                xt_t[:, hh * 1024:(hh + 1) * 1024],
                                xT.ap()[d * 128:(d + 1) * 128,
                                        hh * 1024:(hh + 1) * 1024])
                        xth.append(xt_t)

                    # K/V weights as single strided DMAs (needed at wave 2)
                    nc.sync.dma_start(
                        wk_sb[:].rearrange("p (d c) -> p d c", d=DC),
                        wk.ap().rearrange("(d p) c -> p d c", p=128))
                    nc.sync.dma_start(
                        wv_sb[:].rearrange("p (d c) -> p d c", d=DC),
                        wv.ap().rearrange("(d p) c -> p d c", p=128))
                    nc.sync.dma_start(id_sb[:], ident.ap())
                    nc.sync.dma_start(ones_sb[:], onesd.ap())
                    nc.sync.dma_start(m01_sb[:], m01.ap())

                    # waves 0/1: Q^T for head pairs; wave 2: K^T + V^T.
                    # d-outer order: all 8 PSUM banks accumulate chunk d
                    # before chunk d+1 is touched, so the PE tracks DMA
                    # arrival chunk-by-chunk.
                    for wave in range(3):
                        accs = [pp.tile([128, 512], f32, tag="acc",
                                        name=f"acc_w{wave}_s{s}")
                                for s in range(8)]
                        for d in range(DC):
                            for s in range(8):
                                if wave < 2:
                                    h = wave * 2 + s // 4
                                    lhsT = wqh[d][:, h * HD:(h + 1) * HD]
                                else:
                                    w_sb = wk_sb if s < 4 else wv_sb
                                    lhsT = w_sb[:, d * HD:(d + 1) * HD]
                                t = s % 4
                                nc.tensor.matmul(
                                    accs[s][:], lhsT,
                                    xth[d][:, t * 512:(t + 1) * 512],
                                    start=(d == 0), stop=(d == DC - 1))
                        for s in range(8):
                            if wave < 2:
                                h = wave * 2 + s // 4
                                t = s % 4
                                dst = qt[:, h * N + t * 512:
                                         h * N + (t + 1) * 512]
                            else:
                                src_t = kt if s < 4 else vtmp
                                t = s % 4
                                dst = src_t[:, t * 512:(t + 1) * 512]
                            nc.vector.tensor_copy(dst, accs[s][:])

                    # V^T -> V natural via PE transpose
                    for j in range(NT):
                        tp = pp.tile([128, 128], bf16, tag="acc")
                        nc.tensor.transpose(
                            tp[:], vtmp[:, j * 128:(j + 1) * 128], id_sb[:])
                        nc.vector.tensor_copy(vnat[:, j * 128:(j + 1) * 128], tp[:])

            # wo loads (needed from first outproj; emitted after proj DMAs)
            for h in range(HQ):
                nc.sync.dma_start(wo_sb[h][:], wo.ap()[h * 128:(h + 1) * 128, :])

            # ---------------- Phase 2: attention + out-projection ----------------
            with nc.named_scope("attn"):
                with (
                    tc.tile_pool(name="pt", bufs=4) as pt_pool,
                    tc.tile_pool(name="rr", bufs=2) as rr_pool,
                    tc.tile_pool(name="stage", bufs=4) as stage_pool,
                    tc.tile_pool(name="st", bufs=3, space="PSUM") as st_pool,
                    tc.tile_pool(name="op", bufs=2, space="PSUM") as op_pool,
                    tc.tile_pool(name="sums", bufs=1, space="PSUM") as sums_pool,
                    tc.tile_pool(name="ot", bufs=2, space="PSUM") as ot_pool,
                ):
                    # flat block list across the whole attention phase
                    blocks = [(I, h, j)
                              for I in range(NB)
                              for h in range(HQ)
                              for j in range(4 * I + 4)]

                    def emit_S(I, h, j):
                        """Scores matmul + exp (+ diagonal mask) for one
                        128-wide k block.  Returns the P^T tile + qlo."""
                        o = j - 4 * I
                        qlo = max(0, o) * 128
                        stp = st_pool.tile([128, BW], f32, tag="st")
                        nc.tensor.matmul(
                            stp[:, qlo:],
                            kt[:, j * 128:(j + 1) * 128],
                            qt[:, h * N + I * BW + qlo:
                               h * N + (I + 1) * BW])
                        ptp = pt_pool.tile([128, BW], bf16, tag="pt")
                        nc.scalar.activation(
                            ptp[:, qlo:], stp[:, qlo:], EXP, scale=SCALE)
                        if o >= 0:
                            # triangular boundary within first 128 cols
                            nc.vector.tensor_mul(
                                ptp[:, qlo:qlo + 128],
                                ptp[:, qlo:qlo + 128],
                                m01_sb[:, 0:128])
                        return ptp, qlo

                    def emit_outproj(I):
                        src = ot_sb2[I % 2]
                        for t in range(4):
                            for half in range(2):
                                stg = stage_pool.tile([128, 1024], bf16,
                                                      tag="stg")
                                for u in range(2):
                                    dt = half * 2 + u
                                    opp = op_pool.tile([128, 512], f32,
                                                       tag="op")
                                    for h in range(HQ):
                                        nc.tensor.matmul(
                                            opp[:],
                                            src[:, h * BW + t * 128:
                                                h * BW + (t + 1) * 128],
                                            wo_sb[h][:, dt * 512:
                                                      (dt + 1) * 512],
                                            start=(h == 0),
                                            stop=(h == HQ - 1))
                                    nc.vector.tensor_copy(
                                        stg[:, u * 512:(u + 1) * 512], opp[:])
                                nc.sync.dma_start(
                                    out.ap()[I * BW + t * 128:
                                             I * BW + (t + 1) * 128,
                                             half * 1024:(half + 1) * 1024],
                                    stg[:])

                    # software pipeline: S-parts run LA blocks ahead of the
                    # rowsum/O matmuls; outproj of band I is emitted two
                    # blocks into band I+1.
                    sq = {}          # idx -> (ptp, qlo)
                    k_emit = 0
                    cur = {}         # live smp/otp per (I, h)
                    pending_outproj = None   # (band, emit_at_idx)
                    for idx, (I, h, j) in enumerate(blocks):
                        while k_emit < len(blocks) and k_emit <= idx + LA:
                            bI, bh, bj = blocks[k_emit]
                            sq[k_emit] = emit_S(bI, bh, bj)
                            k_emit += 1
                        if pending_outproj is not None and \
                                idx >= pending_outproj[1]:
                            emit_outproj(pending_outproj[0])
                            pending_outproj = None
                        jmax = 4 * I + 3
                        if j == 0:
                            cur[(I, h)] = (
                                sums_pool.tile([128, BW], f32, tag="sums",
                                               name=f"smp_{I}_{h}"),
                                ot_pool.tile([128, BW], f32, tag="ot",
                                             name=f"otp_{I}_{h}"))
                        smp, otp = cur[(I, h)]
                        ptp, qlo = sq.pop(idx)
                        pslice = ptp[:, qlo:]
                        nc.tensor.matmul(
                            smp[:, qlo:], ones_sb[:], pslice,
                            start=(j == 0), stop=(j == jmax))
                        nc.tensor.matmul(
                            otp[:, qlo:], vnat[:, j * 128:(j + 1) * 128],
                            pslice,
                            start=(j == 0), stop=(j == jmax))
                        if j == jmax:
                            # normalize: O^T * (1/sums), all [128, BW] DVE ops
                            rb_sb = rr_pool.tile([128, BW], f32, tag="rb")
                            nc.vector.reciprocal_approx_fast(rb_sb[:], smp[:])
                            nc.vector.tensor_mul(
                                ot_sb2[I % 2][:, h * BW:(h + 1) * BW],
                                otp[:], rb_sb[:])
                            del cur[(I, h)]
                            if h == HQ - 1:
                                pending_outproj = (I, idx + 2)
                    if pending_outproj is not None:
                        emit_outproj(pending_outproj[0])

    nc.compile()
    return nc


def _get_nc():
    if "nc" not in _cache:
        _cache["nc"] = _build()
    return _cache["nc"]


def _host_inputs(x, Wq, Wk, Wv, Wo):
    import ml_dtypes
    bf = ml_dtypes.bfloat16
    x = np.asarray(x, dtype=np.float32)
    Wq = np.asarray(Wq, dtype=bf)
    Wk = np.asarray(Wk, dtype=bf)
    Wv = np.asarray(Wv, dtype=bf)
    Wo = np.asarray(Wo, dtype=bf)

    kk = np.arange(128)[:, None]
    qq = np.arange(128)[None, :]
    m01 = (qq >= kk).astype(bf)
    ident = np.eye(128, dtype=bf)
    onesd = np.ones((128, 128), dtype=bf)

    xTb = [np.ascontiguousarray(x[b].T).astype(bf) for b in range(B)]
    in_maps = []
    for c in range(8):
        b, g = c // 4, c % 4
        in_maps.append({
            "xT": xTb[b],
            "wq": np.ascontiguousarray(Wq[:, g * 512:(g + 1) * 512]),
            "wk": np.ascontiguousarray(Wk[:, g * HD:(g + 1) * HD]),
            "wv": np.ascontiguousarray(Wv[:, g * HD:(g + 1) * HD]),
            "wo": np.ascontiguousarray(Wo[g * 512:(g + 1) * 512, :]),
            "m01": m01,
            "ident": ident,
            "onesd": onesd,
        })
    return in_maps


def run(x, mask, Wq, Wk, Wv, Wo, trace=False, trace_cores=None):
    from concourse.bass_utils import run_bass_kernel_spmd

    nc = _get_nc()
    in_maps = _host_inputs(x, Wq, Wk, Wv, Wo)
    res = run_bass_kernel_spmd(
        nc, in_maps, core_ids=list(range(8)), trace=trace,
        trace_cores=trace_cores)
    full = np.empty((B, N, DIM), dtype=np.float32)
    for b in range(B):
        acc = res.results[b * 4 + 0]["out"].astype(np.float32)
        for g in range(1, 4):
            acc += res.results[b * 4 + g]["out"].astype(np.float32)
        full[b] = acc
    return full, res


def kernel(x, mask, Wq, Wk, Wv, Wo):
    out, _ = run(x, mask, Wq, Wk, Wv, Wo, trace=False)
    return out


# revision 9
# speedup vs baseline: 1.0597x; 1.0029x over previous
"""GQA causal attention kernel for 8 Trainium2 NeuronCores.

Sharding: data-parallel over batch (2) x tensor-parallel over head groups (4).
Core c handles batch b = c // 4 and head group g = c % 4 (query heads
4g..4g+3, KV head g, Wo rows 512g..512(g+1)).  Each core computes a full
[N, DIM] partial of the output projection (bf16); the host sums the 4
partials per batch in fp32.

Matmuls run in bf16 (fp32r measured ~2 cycles/row on HW, bf16 1): inputs are
converted on the host; all PSUM accumulation stays fp32.

Per-core pipeline:
  1. QKV projections from host-pretransposed x^T, d-outer accumulation order
     (8 open PSUM banks per wave) so the PE consumes x^T/W chunks in DMA
     arrival order instead of head-of-line blocking on the full depth.
  2. V^T -> V via PE transposes.
  3. Attention is software-pipelined at 128-row k-block granularity: the
     scores matmul for block j+3 issues on the PE while exp(j) runs on the
     Scalar engine, so the PE never waits for the softmax chain.  Scores are
     computed TRANSPOSED (S^T[k, q]); softmax denominators accumulate in
     broadcast form via an all-ones stationary matmul; O^T accumulates over
     k blocks in PSUM.
  4. Output projection consumes O^T directly as the stationary operand; its
     emission is delayed two blocks into the next band so the PE has work
     while the last head's normalization (DVE) completes.
"""

import numpy as np

B, N, DIM = 2, 2048, 2048
H, KVH, HD = 16, 4, 128
HQ = H // KVH          # query heads per core
SCALE = float(HD) ** -0.5
NT = N // 128          # 16 seq tiles
DC = DIM // 128        # 16 contraction chunks
NB = 4                 # q bands of 512
BW = N // NB           # 512 band width
LA = 3                 # attention block lookahead (PE pipeline depth)

_cache = {}


def _build():
    import concourse.bass as bass
    import concourse.bacc as bacc
    import concourse.tile as tile
    import concourse.mybir as mybir

    f32 = mybir.dt.float32
    bf16 = mybir.dt.bfloat16
    EXP = mybir.ActivationFunctionType.Exp

    nc = bacc.Bacc("TRN2", target_bir_lowering=False, debug=False)

    xT = nc.dram_tensor("xT", [DIM, N], bf16, kind="ExternalInput")
    wq = nc.dram_tensor("wq", [DIM, HQ * HD], bf16, kind="ExternalInput")
    wk = nc.dram_tensor("wk", [DIM, HD], bf16, kind="ExternalInput")
    wv = nc.dram_tensor("wv", [DIM, HD], bf16, kind="ExternalInput")
    wo = nc.dram_tensor("wo", [HQ * HD, DIM], bf16, kind="ExternalInput")
    m01 = nc.dram_tensor("m01", [128, 128], bf16, kind="ExternalInput")
    ident = nc.dram_tensor("ident", [128, 128], bf16, kind="ExternalInput")
    onesd = nc.dram_tensor("onesd", [128, 128], bf16, kind="ExternalInput")
    out = nc.dram_tensor("out", [N, DIM], bf16, kind="ExternalOutput")

    with tile.TileContext(nc) as tc:
        from contextlib import ExitStack

        with ExitStack() as ctx:
            resident = ctx.enter_context(tc.tile_pool(name="resident", bufs=1))

            # --- resident tiles ---
            qt = resident.tile([128, HQ * N], bf16)        # Q^T all heads
            kt = resident.tile([128, N], bf16)             # K^T
            vnat = resident.tile([128, N], bf16)           # V (seq-major chunks)
            m01_sb = resident.tile([128, 128], bf16)
            id_sb = resident.tile([128, 128], bf16)
            ones_sb = resident.tile([128, 128], bf16)
            wo_sb = [resident.tile([128, DIM], bf16, tag=f"wo{h}", name=f"wo{h}")
                     for h in range(HQ)]
            ot_sb2 = [resident.tile([128, HQ * BW], bf16, tag=f"ot_sb{i}",
                                    name=f"ot_sb{i}") for i in range(2)]

            # ---------------- Phase 1: projections ----------------
            with nc.named_scope("proj"):
                with (
                    tc.tile_pool(name="xth", bufs=1) as xth_pool,
                    tc.tile_pool(name="wqh", bufs=1) as wqh_pool,
                    tc.tile_pool(name="wkv", bufs=1) as wkv_pool,
                    tc.tile_pool(name="pp", bufs=8, space="PSUM") as pp,
                    tc.tile_pool(name="vt", bufs=1) as vt_pool,
                ):
                    # PE warmup while input DMAs land: matmul activity flips
                    # the HAM clock gate to 8/8 before real work starts.
                    # Reads uninitialized SBUF (values unused); sink DMA keeps
                    # the chain from being dead-code-eliminated.
                    warm = pp.tile([128, 512], f32, tag="acc")
                    for _ in range(12):
                        nc.tensor.matmul(warm[:], ones_sb[:], kt[:, 0:512])
                    wsink = vt_pool.tile([128, 512], f32, tag="wsink")
                    nc.vector.tensor_copy(wsink[:], warm[:])
                    sink_dram = nc.dram_tensor("warm_sink", [128, 512], f32,
                                               kind="Internal")
                    nc.sync.dma_start(sink_dram.ap(), wsink[:])
                    wk_sb = wkv_pool.tile([128, DC * HD], bf16, tag="wk")
                    wv_sb = wkv_pool.tile([128, DC * HD], bf16, tag="wv")
                    vtmp = vt_pool.tile([128, N], bf16)    # V^T before transpose

                    # x^T / Wq chunks first (first matmuls need them); each
                    # x^T chunk split in two so transfers spread across DMA
                    # queues and matmuls start on the first half.
                    xth = []
                    wqh = []
                    for d in range(DC):
                        wq_t = wqh_pool.tile([128, HQ * HD], bf16, tag=f"w{d}",
                                             name=f"wqh{d}")
                        nc.sync.dma_start(
                            wq_t[:], wq.ap()[d * 128:(d + 1) * 128, :])
                        wqh.append(wq_t)
                        xt_t = xth_pool.tile([128, N], bf16, tag=f"x{d}",
                                             name=f"xth{d}")
                        for hh in range(2):
                            nc.sync.dma_start(
                                xt_t[:, hh * 1024:(hh + 1) * 1024],
                                xT.ap()[d * 128:(d + 1) * 128,
                                        hh * 1024:(hh + 1) * 1024])
                        xth.append(xt_t)

                    # K/V weights as single strided DMAs (needed at wave 2)
                    nc.sync.dma_start(
                        wk_sb[:].rearrange("p (d c) -> p d c", d=DC),
                        wk.ap().rearrange("(d p) c -> p d c", p=128))
                    nc.sync.dma_start(
                        wv_sb[:].rearrange("p (d c) -> p d c", d=DC),
                        wv.ap().rearrange("(d p) c -> p d c", p=128))
                    nc.sync.dma_start(id_sb[:], ident.ap())
                    nc.sync.dma_start(ones_sb[:], onesd.ap())
                    nc.sync.dma_start(m01_sb[:], m01.ap())

                    # PSUM evacuation copies alternate DVE / Scalar so a
                    # wave's 8 copies drain ~2x faster than on DVE alone
                    # (the next wave's matmuls recycle the banks; GpSimd
                    # cannot access PSUM).
                    COPY = mybir.ActivationFunctionType.Copy

                    def evac(dst, src, s):
                        if s % 2 == 0:
                            nc.vector.tensor_copy(dst, src)
                        else:
                            nc.scalar.activation(dst, src, COPY)

                    # waves 0/1: Q^T for head pairs; wave 2: V^T (s<4, so
                    # the transposes right after the wave get fed first)
                    # + K^T (s>=4).  d-outer order: all 8 PSUM banks
                    # accumulate chunk d before chunk d+1 is touched, so
                    # the PE tracks DMA arrival chunk-by-chunk.
                    for wave in range(3):
                        accs = [pp.tile([128, 512], f32, tag="acc",
                                        name=f"acc_w{wave}_s{s}")
                                for s in range(8)]
                        for d in range(DC):
                            for s in range(8):
                                if wave < 2:
                                    h = wave * 2 + s // 4
                                    lhsT = wqh[d][:, h * HD:(h + 1) * HD]
                                else:
                                    w_sb = wv_sb if s < 4 else wk_sb
                                    lhsT = w_sb[:, d * HD:(d + 1) * HD]
                                t = s % 4
                                nc.tensor.matmul(
                                    accs[s][:], lhsT,
                                    xth[d][:, t * 512:(t + 1) * 512],
                                    start=(d == 0), stop=(d == DC - 1))
                        for s in range(8):
                            if wave < 2:
                                h = wave * 2 + s // 4
                                t = s % 4
                                dst = qt[:, h * N + t * 512:
                                         h * N + (t + 1) * 512]
                            else:
                                src_t = vtmp if s < 4 else kt
                                t = s % 4
                                dst = src_t[:, t * 512:(t + 1) * 512]
                            evac(dst, accs[s][:], s)

                    # V^T -> V natural via PE transpose
                    for j in range(NT):
                        tp = pp.tile([128, 128], bf16, tag="acc")
                        nc.tensor.transpose(
                            tp[:], vtmp[:, j * 128:(j + 1) * 128], id_sb[:])
                        evac(vnat[:, j * 128:(j + 1) * 128], tp[:], j)

            # wo loads (needed from first outproj; emitted after proj DMAs)
            for h in range(HQ):
                nc.sync.dma_start(wo_sb[h][:], wo.ap()[h * 128:(h + 1) * 128, :])

            # ---------------- Phase 2: attention + out-projection ----------------
            with nc.named_scope("attn"):
                with (
                    tc.tile_pool(name="pt", bufs=4) as pt_pool,
                    tc.tile_pool(name="rr", bufs=2) as rr_pool,
                    tc.tile_pool(name="stage", bufs=4) as stage_pool,
                    tc.tile_pool(name="st", bufs=3, space="PSUM") as st_pool,
                    tc.tile_pool(name="op", bufs=2, space="PSUM") as op_pool,
                    tc.tile_pool(name="sums", bufs=1, space="PSUM") as sums_pool,
                    tc.tile_pool(name="ot", bufs=2, space="PSUM") as ot_pool,
                ):
                    # flat block list across the whole attention phase
                    blocks = [(I, h, j)
                              for I in range(NB)
                              for h in range(HQ)
                              for j in range(4 * I + 4)]

                    def emit_S(I, h, j):
                        """Scores matmul + exp (+ diagonal mask) for one
                        128-wide k block.  Returns the P^T tile + qlo."""
                        o = j - 4 * I
                        qlo = max(0, o) * 128
                        stp = st_pool.tile([128, BW], f32, tag="st")
                        nc.tensor.matmul(
                            stp[:, qlo:],
                            kt[:, j * 128:(j + 1) * 128],
                            qt[:, h * N + I * BW + qlo:
                               h * N + (I + 1) * BW])
                        ptp = pt_pool.tile([128, BW], bf16, tag="pt")
                        nc.scalar.activation(
                            ptp[:, qlo:], stp[:, qlo:], EXP, scale=SCALE)
                        if o >= 0:
                            # triangular boundary within first 128 cols;
                            # on Pool so it never queues behind DVE norm ops
                            nc.gpsimd.tensor_mul(
                                ptp[:, qlo:qlo + 128],
                                ptp[:, qlo:qlo + 128],
                                m01_sb[:, 0:128])
                        return ptp, qlo

                    def emit_outproj(I):
                        src = ot_sb2[I % 2]
                        for t in range(4):
                            for half in range(2):
                                stg = stage_pool.tile([128, 1024], bf16,
                                                      tag="stg")
                                for u in range(2):
                                    dt = half * 2 + u
                                    opp = op_pool.tile([128, 512], f32,
                                                       tag="op")
                                    for h in range(HQ):
                                        nc.tensor.matmul(
                                            opp[:],
                                            src[:, h * BW + t * 128:
                                                h * BW + (t + 1) * 128],
                                            wo_sb[h][:, dt * 512:
                                                      (dt + 1) * 512],
                                            start=(h == 0),
                                            stop=(h == HQ - 1))
                                    nc.vector.tensor_copy(
                                        stg[:, u * 512:(u + 1) * 512], opp[:])
                                nc.sync.dma_start(
                                    out.ap()[I * BW + t * 128:
                                             I * BW + (t + 1) * 128,
                                             half * 1024:(half + 1) * 1024],
                                    stg[:])

                    # software pipeline: S-parts run LA blocks ahead of the
                    # rowsum/O matmuls; outproj of band I is emitted two
                    # blocks into band I+1.
                    sq = {}          # idx -> (ptp, qlo)
                    k_emit = 0
                    cur = {}         # live smp/otp per (I, h)
                    pending_outproj = None   # (band, emit_at_idx)
                    for idx, (I, h, j) in enumerate(blocks):
                        while k_emit < len(blocks) and k_emit <= idx + LA:
                            bI, bh, bj = blocks[k_emit]
                            sq[k_emit] = emit_S(bI, bh, bj)
                            k_emit += 1
                        if pending_outproj is not None and \
                                idx >= pending_outproj[1]:
                            emit_outproj(pending_outproj[0])
                            pending_outproj = None
                        jmax = 4 * I + 3
                        if j == 0:
                            cur[(I, h)] = (
                                sums_pool.tile([128, BW], f32, tag="sums",
                                               name=f"smp_{I}_{h}"),
                                ot_pool.tile([128, BW], f32, tag="ot",
                                             name=f"otp_{I}_{h}"))
                        smp, otp = cur[(I, h)]
                        ptp, qlo = sq.pop(idx)
                        pslice = ptp[:, qlo:]
                        nc.tensor.matmul(
                            smp[:, qlo:], ones_sb[:], pslice,
                            start=(j == 0), stop=(j == jmax))
                        nc.tensor.matmul(
                            otp[:, qlo:], vnat[:, j * 128:(j + 1) * 128],
                            pslice,
                            start=(j == 0), stop=(j == jmax))
                        if j == jmax:
                            # normalize: O^T * (1/sums), all [128, BW] DVE ops
                            rb_sb = rr_pool.tile([128, BW], f32, tag="rb")
                            nc.vector.reciprocal_approx_fast(rb_sb[:], smp[:])
                            nc.vector.tensor_mul(
                                ot_sb2[I % 2][:, h * BW:(h + 1) * BW],
                                otp[:], rb_sb[:])
                            del cur[(I, h)]
                            if h == HQ - 1:
                                pending_outproj = (I, idx + 2)
                    if pending_outproj is not None:
                        emit_outproj(pending_outproj[0])

    nc.compile()
    return nc


def _get_nc():
    if "nc" not in _cache:
        _cache["nc"] = _build()
    return _cache["nc"]


def _host_inputs(x, Wq, Wk, Wv, Wo):
    import ml_dtypes
    bf = ml_dtypes.bfloat16
    x = np.asarray(x, dtype=np.float32)
    Wq = np.asarray(Wq, dtype=bf)
    Wk = np.asarray(Wk, dtype=bf)
    Wv = np.asarray(Wv, dtype=bf)
    Wo = np.asarray(Wo, dtype=bf)

    kk = np.arange(128)[:, None]
    qq = np.arange(128)[None, :]
    m01 = (qq >= kk).astype(bf)
    ident = np.eye(128, dtype=bf)
    onesd = np.ones((128, 128), dtype=bf)

    xTb = [np.ascontiguousarray(x[b].T).astype(bf) for b in range(B)]
    in_maps = []
    for c in range(8):
        b, g = c // 4, c % 4
        in_maps.append({
            "xT": xTb[b],
            "wq": np.ascontiguousarray(Wq[:, g * 512:(g + 1) * 512]),
            "wk": np.ascontiguousarray(Wk[:, g * HD:(g + 1) * HD]),
            "wv": np.ascontiguousarray(Wv[:, g * HD:(g + 1) * HD]),
            "wo": np.ascontiguousarray(Wo[g * 512:(g + 1) * 512, :]),
            "m01": m01,
            "ident": ident,
            "onesd": onesd,
        })
    return in_maps


def run(x, mask, Wq, Wk, Wv, Wo, trace=False, trace_cores=None):
    from concourse.bass_utils import run_bass_kernel_spmd

    nc = _get_nc()
    in_maps = _host_inputs(x, Wq, Wk, Wv, Wo)
    res = run_bass_kernel_spmd(
        nc, in_maps, core_ids=list(range(8)), trace=trace,
        trace_cores=trace_cores)
    full = np.empty((B, N, DIM), dtype=np.float32)
    for b in range(B):
        acc = res.results[b * 4 + 0]["out"].astype(np.float32)
        for g in range(1, 4):
            acc += res.results[b * 4 + g]["out"].astype(np.float32)
        full[b] = acc
    return full, res


def kernel(x, mask, Wq, Wk, Wv, Wo):
    out, _ = run(x, mask, Wq, Wk, Wv, Wo, trace=False)
    return out


# revision 36
# speedup vs baseline: 1.0860x; 1.0248x over previous
"""GQA causal attention kernel for 8 Trainium2 NeuronCores.

Sharding: data-parallel over batch (2) x tensor-parallel over head groups (4).
Core c handles batch b = c // 4 and head group g = c % 4 (query heads
4g..4g+3, KV head g, Wo rows 512g..512(g+1)).  Each core computes a full
[N, DIM] partial of the output projection (bf16); the host sums the 4
partials per batch in fp32.

Matmuls run in bf16 (fp32r measured ~2 cycles/row on HW, bf16 1): inputs are
converted on the host; all PSUM accumulation stays fp32.

Per-core pipeline:
  1. QKV projections from host-packed [Wq_d | x^T_d] chunks (one 5KB-line
     DMA per chunk), d-outer accumulation order (8 open PSUM banks per wave)
     so the PE consumes chunks in DMA arrival order instead of head-of-line
     blocking on the full depth.
  2. V^T -> V via PE transposes, 4 packed per PSUM tile.
  3. Attention is software-pipelined at 128-row k-block granularity: the
     scores matmul for block j+3 issues on the PE while exp(j) runs on the
     Scalar engine, so the PE never waits for the softmax chain.  Scores are
     computed TRANSPOSED (S^T[k, q]); softmax denominators accumulate in
     broadcast form via an all-ones stationary matmul; O^T accumulates over
     k blocks in PSUM.  Bands run in order 1,2,3,0 (band 0's tiny heads are
     sync-heavy and go last).
  4. Output projection consumes O^T directly as the stationary operand; its
     groups are spread one-per-two-blocks through the next band's loop so
     every exp chain has outproj matmuls as PE cover.
"""

import numpy as np

B, N, DIM = 2, 2048, 2048
H, KVH, HD = 16, 4, 128
HQ = H // KVH          # query heads per core
SCALE = float(HD) ** -0.5
NT = N // 128          # 16 seq tiles
DC = DIM // 128        # 16 contraction chunks
NB = 4                 # q bands of 512
BW = N // NB           # 512 band width
LA = 3                 # attention block lookahead (PE pipeline depth)
CW = 512 + N           # packed [Wq_d | x^T_d] columns per chunk

_cache = {}


def _build():
    import concourse.bass as bass
    import concourse.bacc as bacc
    import concourse.tile as tile
    import concourse.mybir as mybir

    f32 = mybir.dt.float32
    bf16 = mybir.dt.bfloat16
    EXP = mybir.ActivationFunctionType.Exp

    nc = bacc.Bacc("TRN2", target_bir_lowering=False, debug=False)

    # xw packs, per 128-deep contraction chunk d, [Wq_d (512) | x^T_d (2048)]
    # so one chunk = one or two large-line DMAs.  wkv packs Wk/Wv chunks
    # contiguously ([p, d*HD+c] = W[d*128+p, c]); consts packs m01|ident|ones.
    xw = nc.dram_tensor("xw", [128, DC * CW], bf16, kind="ExternalInput")
    wkv = nc.dram_tensor("wkv", [128, 2 * DC * HD], bf16,
                         kind="ExternalInput")
    wo = nc.dram_tensor("wo", [HQ * HD, DIM], bf16, kind="ExternalInput")
    consts = nc.dram_tensor("consts", [128, 384], bf16, kind="ExternalInput")
    out = nc.dram_tensor("out", [N, DIM], bf16, kind="ExternalOutput")

    with tile.TileContext(nc) as tc:
        from contextlib import ExitStack

        with ExitStack() as ctx:
            resident = ctx.enter_context(tc.tile_pool(name="resident", bufs=1))

            # --- resident tiles ---
            qt = resident.tile([128, HQ * N], bf16)        # Q^T all heads
            kt = resident.tile([128, N], bf16)             # K^T
            vnat = resident.tile([128, N], bf16)           # V (seq-major chunks)
            wo_sb = [resident.tile([128, DIM], bf16, tag=f"wo{h}", name=f"wo{h}")
                     for h in range(HQ)]
            ot_sb2 = [resident.tile([128, HQ * BW], bf16, tag=f"ot_sb{i}",
                                    name=f"ot_sb{i}") for i in range(2)]
            cst = resident.tile([128, 384], bf16, name="cst")
            m01_sb = cst[:, 0:128]
            id_sb = cst[:, 128:256]
            ones_sb = cst[:, 256:384]

            # ---------------- Phase 1: projections ----------------
            with nc.named_scope("proj"):
                with (
                    tc.tile_pool(name="xth", bufs=1) as xth_pool,
                    tc.tile_pool(name="wqh", bufs=1) as wqh_pool,
                    tc.tile_pool(name="wkv", bufs=1) as wkv_pool,
                    tc.tile_pool(name="pp", bufs=8, space="PSUM") as pp,
                    tc.tile_pool(name="vt", bufs=1) as vt_pool,
                ):
                    # PE warmup while input DMAs land: matmul activity flips
                    # the HAM clock gate to 8/8 before real work starts.
                    # Reads uninitialized SBUF (values unused); sink DMA keeps
                    # the chain from being dead-code-eliminated.
                    warm = pp.tile([128, 512], f32, tag="acc")
                    for _ in range(16):
                        nc.tensor.matmul(warm[:], ones_sb[:], kt[:, 0:512])
                    wsink = vt_pool.tile([128, 512], f32, tag="wsink")
                    nc.vector.tensor_copy(wsink[:], warm[:])
                    sink_dram = nc.dram_tensor("warm_sink", [128, 512], f32,
                                               kind="Internal")
                    nc.sync.dma_start(sink_dram.ap(), wsink[:])
                    wkv_sb = wkv_pool.tile([128, 2 * DC * HD], bf16, tag="wkv")
                    wk_sb = wkv_sb[:, 0:DC * HD]
                    wv_sb = wkv_sb[:, DC * HD:]
                    vtmp = vt_pool.tile([128, N], bf16)    # V^T before transpose

                    # packed [Wq_d | x^T_d] chunks; one 5KB-line DMA per
                    # chunk (arrival ~1.6us < PE consumption ~1.7us, so the
                    # pipeline stays PE-bound once the first chunk lands).
                    xth = []
                    wqh = []
                    for d in range(DC):
                        ck = xth_pool.tile([128, CW], bf16, tag=f"x{d}",
                                           name=f"xw{d}")
                        nc.sync.dma_start(
                            ck[:], xw.ap()[:, d * CW:(d + 1) * CW])
                        wqh.append(ck[:, 0:HQ * HD])
                        xth.append(ck[:, HQ * HD:])

                    # K/V weights (needed at wave 2) + constants, one DMA each
                    nc.sync.dma_start(wkv_sb[:], wkv.ap())
                    nc.sync.dma_start(cst[:], consts.ap())

                    # PSUM evacuation copies alternate DVE / Scalar so a
                    # wave's 8 copies drain ~2x faster than on DVE alone
                    # (the next wave's matmuls recycle the banks; GpSimd
                    # cannot access PSUM).
                    COPY = mybir.ActivationFunctionType.Copy

                    def evac(dst, src, s):
                        if s % 2 == 0:
                            nc.vector.tensor_copy(dst, src)
                        else:
                            nc.scalar.activation(dst, src, COPY)

                    # waves 0/1: Q^T for head pairs; wave 2: V^T (s<4, so
                    # the transposes right after the wave get fed first)
                    # + K^T (s>=4).  d-outer order: all 8 PSUM banks
                    # accumulate chunk d before chunk d+1 is touched, so
                    # the PE tracks DMA arrival chunk-by-chunk.
                    for wave in range(3):
                        accs = [pp.tile([128, 512], f32, tag="acc",
                                        name=f"acc_w{wave}_s{s}")
                                for s in range(8)]
                        for d in range(DC):
                            for s in range(8):
                                if wave < 2:
                                    h = wave * 2 + s // 4
                                    lhsT = wqh[d][:, h * HD:(h + 1) * HD]
                                else:
                                    w_sb = wv_sb if s < 4 else wk_sb
                                    lhsT = w_sb[:, d * HD:(d + 1) * HD]
                                t = s % 4
                                nc.tensor.matmul(
                                    accs[s][:], lhsT,
                                    xth[d][:, t * 512:(t + 1) * 512],
                                    start=(d == 0), stop=(d == DC - 1))
                        if wave < 2:
                            for s in range(8):
                                h = wave * 2 + s // 4
                                t = s % 4
                                evac(qt[:, h * N + t * 512:
                                        h * N + (t + 1) * 512],
                                     accs[s][:], s)
                        else:
                            # kt first on DVE, vtmp first on Scalar: the
                            # first attention scores (need kt) and the
                            # transposes (need vtmp) both start ~0.8us
                            # after the wave
                            for i, s in enumerate((4, 0, 5, 1, 6, 2, 7, 3)):
                                src_t = vtmp if s < 4 else kt
                                t = s % 4
                                evac(src_t[:, t * 512:(t + 1) * 512],
                                     accs[s][:], i)

                    # V^T -> V natural via PE transpose, 4 transposes packed
                    # per PSUM tile so only 4 tiles + 4 wide evacs stand
                    # between the wave-2 banks and the attention score tiles
                    for g in range(4):
                        tp = pp.tile([128, 512], bf16, tag="acc",
                                     name=f"tp{g}")
                        for u in range(4):
                            j = g * 4 + u
                            nc.tensor.transpose(
                                tp[:, u * 128:(u + 1) * 128],
                                vtmp[:, j * 128:(j + 1) * 128], id_sb[:])
                        evac(vnat[:, g * 512:(g + 1) * 512], tp[:], g)

            # wo loads (needed from first outproj; emitted after proj DMAs)
            for h in range(HQ):
                nc.sync.dma_start(wo_sb[h][:], wo.ap()[h * 128:(h + 1) * 128, :])

            # ---------------- Phase 2: attention + out-projection ----------------
            with nc.named_scope("attn"):
                with (
                    tc.tile_pool(name="pt", bufs=4) as pt_pool,
                    tc.tile_pool(name="rr", bufs=2) as rr_pool,
                    tc.tile_pool(name="stage", bufs=4) as stage_pool,
                    tc.tile_pool(name="st", bufs=3, space="PSUM") as st_pool,
                    tc.tile_pool(name="op", bufs=2, space="PSUM") as op_pool,
                    tc.tile_pool(name="sums", bufs=1, space="PSUM") as sums_pool,
                    tc.tile_pool(name="ot", bufs=2, space="PSUM") as ot_pool,
                ):
                    # flat block list across the whole attention phase.
                    # Band 0 (tiny 4-block heads, sync-heavy) goes last so
                    # the proj->attn transition starts on band 1's larger
                    # blocks, which keep the PE pipeline full.
                    blocks = [(I, h, j)
                              for I in (1, 2, 3, 0)
                              for h in range(HQ)
                              for j in range(4 * I + 4)]

                    def emit_S(I, h, j, pool=None, tag="st"):
                        """Scores matmul + exp (+ diagonal mask) for one
                        128-wide k block.  Returns the P^T tile + qlo."""
                        o = j - 4 * I
                        qlo = max(0, o) * 128
                        stp = (pool or st_pool).tile([128, BW], f32, tag=tag,
                                                     name=f"st_{I}_{h}_{j}")
                        nc.tensor.matmul(
                            stp[:, qlo:],
                            kt[:, j * 128:(j + 1) * 128],
                            qt[:, h * N + I * BW + qlo:
                               h * N + (I + 1) * BW])
                        ptp = pt_pool.tile([128, BW], bf16, tag="pt")
                        nc.scalar.activation(
                            ptp[:, qlo:], stp[:, qlo:], EXP, scale=SCALE)
                        if o >= 0:
                            # triangular boundary within first 128 cols;
                            # on Pool so it never queues behind DVE norm ops
                            nc.gpsimd.tensor_mul(
                                ptp[:, qlo:qlo + 128],
                                ptp[:, qlo:qlo + 128],
                                m01_sb[:, 0:128])
                        return ptp, qlo

                    def outproj_group(I, t, half):
                        """One outproj group: [128 q, 1024 out] = 2 PSUM
                        tiles (8 matmuls, ~1.7us PE) + 2 copies + 1 DMA."""
                        src = ot_sb2[I % 2]
                        stg = stage_pool.tile([128, 1024], bf16, tag="stg",
                                              name=f"stg{I}_{t}_{half}")
                        for u in range(2):
                            dt = half * 2 + u
                            opp = op_pool.tile([128, 512], f32, tag="op",
                                               name=f"op{I}_{t}_{dt}")
                            for h in range(HQ):
                                nc.tensor.matmul(
                                    opp[:],
                                    src[:, h * BW + t * 128:
                                        h * BW + (t + 1) * 128],
                                    wo_sb[h][:, dt * 512:(dt + 1) * 512],
                                    start=(h == 0), stop=(h == HQ - 1))
                            nc.vector.tensor_copy(
                                stg[:, u * 512:(u + 1) * 512], opp[:])
                        nc.sync.dma_start(
                            out.ap()[I * BW + t * 128:I * BW + (t + 1) * 128,
                                     half * 1024:(half + 1) * 1024],
                            stg[:])

                    # software pipeline: S-parts run LA blocks ahead of the
                    # rowsum/O matmuls; the outproj of band I is spread one
                    # group per block-iteration through the next band
                    # (starting 2 blocks in, so the last head's DVE
                    # normalization has finished), giving every exp chain
                    # ~2us of outproj matmuls as cover.
                    sq = {}          # idx -> (ptp, qlo)
                    k_emit = 0
                    cur = {}         # live smp/otp per (I, h)
                    pend = []        # outproj groups awaiting emission
                    pend_from = 0    # first idx allowed to emit them
                    for idx, (I, h, j) in enumerate(blocks):
                        while k_emit < len(blocks) and \
                                k_emit <= idx + LA + (2 if idx == 0 else 0):
                            bI, bh, bj = blocks[k_emit]
                            if k_emit in (3, 4):
                                # at attention start the op pool is idle (no
                                # outproj pending yet): borrow its 2 banks
                                # for extra score prefetch so the first
                                # head's exp chain has ~1.4us of PE cover
                                sq[k_emit] = emit_S(bI, bh, bj,
                                                    pool=op_pool, tag="op")
                            else:
                                sq[k_emit] = emit_S(bI, bh, bj)
                            k_emit += 1
                        if pend and idx >= pend_from and \
                                (idx - pend_from) % 2 == 0:
                            outproj_group(*pend.pop(0))
                        jmax = 4 * I + 3
                        if j == 0:
                            cur[(I, h)] = (
                                sums_pool.tile([128, BW], f32, tag="sums",
                                               name=f"smp_{I}_{h}"),
                                ot_pool.tile([128, BW], f32, tag="ot",
                                             name=f"otp_{I}_{h}"))
                        smp, otp = cur[(I, h)]
                        ptp, qlo = sq.pop(idx)
                        pslice = ptp[:, qlo:]
                        nc.tensor.matmul(
                            smp[:, qlo:], ones_sb[:], pslice,
                            start=(j == 0), stop=(j == jmax))
                        nc.tensor.matmul(
                            otp[:, qlo:], vnat[:, j * 128:(j + 1) * 128],
                            pslice,
                            start=(j == 0), stop=(j == jmax))
                        if j == jmax:
                            # normalize: O^T * (1/sums), all [128, BW] DVE ops
                            rb_sb = rr_pool.tile([128, BW], f32, tag="rb")
                            nc.vector.reciprocal_approx_fast(rb_sb[:], smp[:])
                            nc.vector.tensor_mul(
                                ot_sb2[I % 2][:, h * BW:(h + 1) * BW],
                                otp[:], rb_sb[:])
                            del cur[(I, h)]
                            if h == HQ - 1:
                                pend += [(I, t, hf)
                                         for t in range(4) for hf in range(2)]
                                pend_from = idx + 2
                    for g in pend:
                        outproj_group(*g)

    nc.compile()
    return nc


def _get_nc():
    if "nc" not in _cache:
        _cache["nc"] = _build()
    return _cache["nc"]


def _host_inputs(x, Wq, Wk, Wv, Wo):
    import ml_dtypes
    bf = ml_dtypes.bfloat16
    x = np.asarray(x, dtype=np.float32)
    Wq = np.asarray(Wq, dtype=bf)
    Wk = np.asarray(Wk, dtype=bf)
    Wv = np.asarray(Wv, dtype=bf)
    Wo = np.asarray(Wo, dtype=bf)

    kk = np.arange(128)[:, None]
    qq = np.arange(128)[None, :]
    m01 = (qq >= kk).astype(bf)
    ident = np.eye(128, dtype=bf)
    onesd = np.ones((128, 128), dtype=bf)
    consts = np.ascontiguousarray(
        np.concatenate([m01, ident, onesd], axis=1))

    xTb = [np.ascontiguousarray(x[b].T).astype(bf) for b in range(B)]
    in_maps = []
    for c in range(8):
        b, g = c // 4, c % 4
        Wq_g = Wq[:, g * 512:(g + 1) * 512]
        xT = xTb[b]
        xw = np.empty((128, DC * CW), dtype=bf)
        wkv = np.empty((128, 2 * DC * HD), dtype=bf)
        for d in range(DC):
            xw[:, d * CW:d * CW + 512] = Wq_g[d * 128:(d + 1) * 128, :]
            xw[:, d * CW + 512:(d + 1) * CW] = xT[d * 128:(d + 1) * 128, :]
            wkv[:, d * HD:(d + 1) * HD] = \
                Wk[d * 128:(d + 1) * 128, g * HD:(g + 1) * HD]
            wkv[:, DC * HD + d * HD:DC * HD + (d + 1) * HD] = \
                Wv[d * 128:(d + 1) * 128, g * HD:(g + 1) * HD]
        in_maps.append({
            "xw": xw,
            "wkv": wkv,
            "wo": np.ascontiguousarray(Wo[g * 512:(g + 1) * 512, :]),
            "consts": consts,
        })
    return in_maps


def run(x, mask, Wq, Wk, Wv, Wo, trace=False, trace_cores=None):
    from concourse.bass_utils import run_bass_kernel_spmd

    nc = _get_nc()
    in_maps = _host_inputs(x, Wq, Wk, Wv, Wo)
    res = run_bass_kernel_spmd(
        nc, in_maps, core_ids=list(range(8)), trace=trace,
        trace_cores=trace_cores)
    full = np.empty((B, N, DIM), dtype=np.float32)
    for b in range(B):
        acc = res.results[b * 4 + 0]["out"].astype(np.float32)
        for g in range(1, 4):
            acc += res.results[b * 4 + g]["out"].astype(np.float32)
        full[b] = acc
    return full, res


def kernel(x, mask, Wq, Wk, Wv, Wo):
    out, _ = run(x, mask, Wq, Wk, Wv, Wo, trace=False)
    return out


# revision 37
# speedup vs baseline: 1.0981x; 1.0112x over previous
"""GQA causal attention kernel for 8 Trainium2 NeuronCores.

Sharding: data-parallel over batch (2) x tensor-parallel over head groups (4).
Core c handles batch b = c // 4 and head group g = c % 4 (query heads
4g..4g+3, KV head g, Wo rows 512g..512(g+1)).  Each core computes a full
[N, DIM] partial of the output projection (bf16); the host sums the 4
partials per batch in fp32.

Matmuls run in bf16 (fp32r measured ~2 cycles/row on HW, bf16 1): inputs are
converted on the host; all PSUM accumulation stays fp32.

Per-core pipeline:
  1. QKV projections from host-packed [Wq_d | x^T_d] chunks (one 5KB-line
     DMA per chunk), d-outer accumulation order (8 open PSUM banks per wave)
     so the PE consumes chunks in DMA arrival order instead of head-of-line
     blocking on the full depth.
  2. V^T -> V via PE transposes, 4 packed per PSUM tile.
  3. Attention is software-pipelined at 128-row k-block granularity: the
     scores matmul for block j+3 issues on the PE while exp(j) runs on the
     Scalar engine, so the PE never waits for the softmax chain.  Scores are
     computed TRANSPOSED (S^T[k, q]); softmax denominators accumulate in
     broadcast form via an all-ones stationary matmul; O^T accumulates over
     k blocks in PSUM.  Bands run in order 1,2,3,0 (band 0's tiny heads are
     sync-heavy and go last).
  4. Output projection consumes O^T directly as the stationary operand; its
     groups are spread one-per-two-blocks through the next band's loop so
     every exp chain has outproj matmuls as PE cover.
"""

import numpy as np

B, N, DIM = 2, 2048, 2048
H, KVH, HD = 16, 4, 128
HQ = H // KVH          # query heads per core
SCALE = float(HD) ** -0.5
NT = N // 128          # 16 seq tiles
DC = DIM // 128        # 16 contraction chunks
NB = 4                 # q bands of 512
BW = N // NB           # 512 band width
LA = 3                 # attention block lookahead (PE pipeline depth)
CW = 512 + N           # packed [Wq_d | x^T_d] columns per chunk

_cache = {}


def _build():
    import concourse.bass as bass
    import concourse.bacc as bacc
    import concourse.tile as tile
    import concourse.mybir as mybir

    f32 = mybir.dt.float32
    bf16 = mybir.dt.bfloat16
    EXP = mybir.ActivationFunctionType.Exp

    nc = bacc.Bacc("TRN2", target_bir_lowering=False, debug=False)

    # xw packs, per 128-deep contraction chunk d, [Wq_d (512) | x^T_d (2048)]
    # so one chunk = one or two large-line DMAs.  wkv packs Wk/Wv chunks
    # contiguously ([p, d*HD+c] = W[d*128+p, c]); consts packs m01|ident|ones.
    xw = nc.dram_tensor("xw", [128, DC * CW], bf16, kind="ExternalInput")
    wkv = nc.dram_tensor("wkv", [128, 2 * DC * HD], bf16,
                         kind="ExternalInput")
    wo = nc.dram_tensor("wo", [HQ * HD, DIM], bf16, kind="ExternalInput")
    consts = nc.dram_tensor("consts", [128, 384], bf16, kind="ExternalInput")
    out = nc.dram_tensor("out", [N, DIM], bf16, kind="ExternalOutput")

    with tile.TileContext(nc) as tc:
        from contextlib import ExitStack

        with ExitStack() as ctx:
            resident = ctx.enter_context(tc.tile_pool(name="resident", bufs=1))

            # --- resident tiles ---
            qt = resident.tile([128, HQ * N], bf16)        # Q^T all heads
            kt = resident.tile([128, N], bf16)             # K^T
            vnat = resident.tile([128, N], bf16)           # V (seq-major chunks)
            wo_sb = [resident.tile([128, DIM], bf16, tag=f"wo{h}", name=f"wo{h}")
                     for h in range(HQ)]
            ot_sb2 = [resident.tile([128, HQ * BW], bf16, tag=f"ot_sb{i}",
                                    name=f"ot_sb{i}") for i in range(2)]
            cst = resident.tile([128, 384], bf16, name="cst")
            m01_sb = cst[:, 0:128]
            id_sb = cst[:, 128:256]
            ones_sb = cst[:, 256:384]

            # ---------------- Phase 1: projections ----------------
            with nc.named_scope("proj"):
                with (
                    tc.tile_pool(name="xth", bufs=1) as xth_pool,
                    tc.tile_pool(name="wqh", bufs=1) as wqh_pool,
                    tc.tile_pool(name="wkv", bufs=1) as wkv_pool,
                    tc.tile_pool(name="pp", bufs=8, space="PSUM") as pp,
                    tc.tile_pool(name="vt", bufs=1) as vt_pool,
                ):
                    # PE warmup while input DMAs land: matmul activity flips
                    # the HAM clock gate to 8/8 before real work starts.
                    # Reads uninitialized SBUF (values unused); sink DMA keeps
                    # the chain from being dead-code-eliminated.
                    warm = pp.tile([128, 512], f32, tag="acc")
                    for _ in range(16):
                        nc.tensor.matmul(warm[:], ones_sb[:], kt[:, 0:512])
                    wsink = vt_pool.tile([128, 512], f32, tag="wsink")
                    nc.vector.tensor_copy(wsink[:], warm[:])
                    sink_dram = nc.dram_tensor("warm_sink", [128, 512], f32,
                                               kind="Internal")
                    nc.sync.dma_start(sink_dram.ap(), wsink[:])
                    wkv_sb = wkv_pool.tile([128, 2 * DC * HD], bf16, tag="wkv")
                    wk_sb = wkv_sb[:, 0:DC * HD]
                    wv_sb = wkv_sb[:, DC * HD:]
                    vtmp = vt_pool.tile([128, N], bf16)    # V^T before transpose

                    # packed [Wq_d | x^T_d] chunks; one 5KB-line DMA per
                    # chunk (arrival ~1.6us < PE consumption ~1.7us, so the
                    # pipeline stays PE-bound once the first chunk lands).
                    xth = []
                    wqh = []
                    for d in range(DC):
                        ck = xth_pool.tile([128, CW], bf16, tag=f"x{d}",
                                           name=f"xw{d}")
                        nc.sync.dma_start(
                            ck[:], xw.ap()[:, d * CW:(d + 1) * CW])
                        wqh.append(ck[:, 0:HQ * HD])
                        xth.append(ck[:, HQ * HD:])

                    # K/V weights (needed at wave 2) + constants, one DMA each
                    nc.sync.dma_start(wkv_sb[:], wkv.ap())
                    nc.sync.dma_start(cst[:], consts.ap())

                    # PSUM evacuation copies alternate DVE / Scalar so a
                    # wave's 8 copies drain ~2x faster than on DVE alone
                    # (the next wave's matmuls recycle the banks; GpSimd
                    # cannot access PSUM).
                    COPY = mybir.ActivationFunctionType.Copy

                    def evac(dst, src, s):
                        if s % 2 == 0:
                            nc.vector.tensor_copy(dst, src)
                        else:
                            nc.scalar.activation(dst, src, COPY)

                    # waves 0/1: Q^T for head pairs; wave 2: V^T (s<4, so
                    # the transposes right after the wave get fed first)
                    # + K^T (s>=4).  d-outer order: all 8 PSUM banks
                    # accumulate chunk d before chunk d+1 is touched, so
                    # the PE tracks DMA arrival chunk-by-chunk.
                    for wave in range(3):
                        accs = [pp.tile([128, 512], f32, tag="acc",
                                        name=f"acc_w{wave}_s{s}")
                                for s in range(8)]
                        for d in range(DC):
                            for s in range(8):
                                if wave < 2:
                                    h = wave * 2 + s // 4
                                    lhsT = wqh[d][:, h * HD:(h + 1) * HD]
                                else:
                                    w_sb = wv_sb if s < 4 else wk_sb
                                    lhsT = w_sb[:, d * HD:(d + 1) * HD]
                                t = s % 4
                                nc.tensor.matmul(
                                    accs[s][:], lhsT,
                                    xth[d][:, t * 512:(t + 1) * 512],
                                    start=(d == 0), stop=(d == DC - 1))
                                if d == DC - 1:
                                    # evac each tile the moment its last
                                    # accumulation lands: the copies overlap
                                    # the tail of the wave, so the banks are
                                    # free ~1.5us earlier for the next wave
                                    # / transposes / attention scores
                                    if wave < 2:
                                        dst = qt[:, (wave * 2 + s // 4) * N
                                                 + t * 512:
                                                 (wave * 2 + s // 4) * N
                                                 + (t + 1) * 512]
                                    else:
                                        src_t = vtmp if s < 4 else kt
                                        dst = src_t[:, t * 512:(t + 1) * 512]
                                    evac(dst, accs[s][:], s)

                    # V^T -> V natural via PE transpose, 4 transposes packed
                    # per PSUM tile so only 4 tiles + 4 wide evacs stand
                    # between the wave-2 banks and the attention score tiles
                    for g in range(4):
                        tp = pp.tile([128, 512], bf16, tag="acc",
                                     name=f"tp{g}")
                        for u in range(4):
                            j = g * 4 + u
                            nc.tensor.transpose(
                                tp[:, u * 128:(u + 1) * 128],
                                vtmp[:, j * 128:(j + 1) * 128], id_sb[:])
                        evac(vnat[:, g * 512:(g + 1) * 512], tp[:], g)

            # wo loads (needed from first outproj; emitted after proj DMAs)
            for h in range(HQ):
                nc.sync.dma_start(wo_sb[h][:], wo.ap()[h * 128:(h + 1) * 128, :])

            # ---------------- Phase 2: attention + out-projection ----------------
            with nc.named_scope("attn"):
                with (
                    tc.tile_pool(name="pt", bufs=4) as pt_pool,
                    tc.tile_pool(name="rr", bufs=2) as rr_pool,
                    tc.tile_pool(name="stage", bufs=4) as stage_pool,
                    tc.tile_pool(name="st", bufs=3, space="PSUM") as st_pool,
                    tc.tile_pool(name="op", bufs=2, space="PSUM") as op_pool,
                    tc.tile_pool(name="sums", bufs=1, space="PSUM") as sums_pool,
                    tc.tile_pool(name="ot", bufs=2, space="PSUM") as ot_pool,
                ):
                    # flat block list across the whole attention phase.
                    # Band 0 (tiny 4-block heads, sync-heavy) goes last so
                    # the proj->attn transition starts on band 1's larger
                    # blocks, which keep the PE pipeline full.
                    blocks = [(I, h, j)
                              for I in (1, 2, 3, 0)
                              for h in range(HQ)
                              for j in range(4 * I + 4)]

                    def emit_S(I, h, j, pool=None, tag="st"):
                        """Scores matmul + exp (+ diagonal mask) for one
                        128-wide k block.  Returns the P^T tile + qlo."""
                        o = j - 4 * I
                        qlo = max(0, o) * 128
                        stp = (pool or st_pool).tile([128, BW], f32, tag=tag,
                                                     name=f"st_{I}_{h}_{j}")
                        nc.tensor.matmul(
                            stp[:, qlo:],
                            kt[:, j * 128:(j + 1) * 128],
                            qt[:, h * N + I * BW + qlo:
                               h * N + (I + 1) * BW])
                        ptp = pt_pool.tile([128, BW], bf16, tag="pt")
                        nc.scalar.activation(
                            ptp[:, qlo:], stp[:, qlo:], EXP, scale=SCALE)
                        if o >= 0:
                            # triangular boundary within first 128 cols;
                            # on Pool so it never queues behind DVE norm ops
                            nc.gpsimd.tensor_mul(
                                ptp[:, qlo:qlo + 128],
                                ptp[:, qlo:qlo + 128],
                                m01_sb[:, 0:128])
                        return ptp, qlo

                    def outproj_group(I, t, half):
                        """One outproj group: [128 q, 1024 out] = 2 PSUM
                        tiles (8 matmuls, ~1.7us PE) + 2 copies + 1 DMA."""
                        src = ot_sb2[I % 2]
                        stg = stage_pool.tile([128, 1024], bf16, tag="stg",
                                              name=f"stg{I}_{t}_{half}")
                        for u in range(2):
                            dt = half * 2 + u
                            opp = op_pool.tile([128, 512], f32, tag="op",
                                               name=f"op{I}_{t}_{dt}")
                            for h in range(HQ):
                                nc.tensor.matmul(
                                    opp[:],
                                    src[:, h * BW + t * 128:
                                        h * BW + (t + 1) * 128],
                                    wo_sb[h][:, dt * 512:(dt + 1) * 512],
                                    start=(h == 0), stop=(h == HQ - 1))
                            nc.vector.tensor_copy(
                                stg[:, u * 512:(u + 1) * 512], opp[:])
                        nc.sync.dma_start(
                            out.ap()[I * BW + t * 128:I * BW + (t + 1) * 128,
                                     half * 1024:(half + 1) * 1024],
                            stg[:])

                    # software pipeline: S-parts run LA blocks ahead of the
                    # rowsum/O matmuls; the outproj of band I is spread one
                    # group per block-iteration through the next band
                    # (starting 2 blocks in, so the last head's DVE
                    # normalization has finished), giving every exp chain
                    # ~2us of outproj matmuls as cover.
                    sq = {}          # idx -> (ptp, qlo)
                    k_emit = 0
                    cur = {}         # live smp/otp per (I, h)
                    pend = []        # outproj groups awaiting emission
                    pend_from = 0    # first idx allowed to emit them
                    for idx, (I, h, j) in enumerate(blocks):
                        while k_emit < len(blocks) and \
                                k_emit <= idx + LA + (2 if idx == 0 else 0):
                            bI, bh, bj = blocks[k_emit]
                            if k_emit in (3, 4):
                                # at attention start the op pool is idle (no
                                # outproj pending yet): borrow its 2 banks
                                # for extra score prefetch so the first
                                # head's exp chain has ~1.4us of PE cover
                                sq[k_emit] = emit_S(bI, bh, bj,
                                                    pool=op_pool, tag="op")
                            else:
                                sq[k_emit] = emit_S(bI, bh, bj)
                            k_emit += 1
                        if pend and idx >= pend_from and \
                                (idx - pend_from) % 2 == 0:
                            outproj_group(*pend.pop(0))
                        jmax = 4 * I + 3
                        if j == 0:
                            cur[(I, h)] = (
                                sums_pool.tile([128, BW], f32, tag="sums",
                                               name=f"smp_{I}_{h}"),
                                ot_pool.tile([128, BW], f32, tag="ot",
                                             name=f"otp_{I}_{h}"))
                        smp, otp = cur[(I, h)]
                        ptp, qlo = sq.pop(idx)
                        pslice = ptp[:, qlo:]
                        nc.tensor.matmul(
                            smp[:, qlo:], ones_sb[:], pslice,
                            start=(j == 0), stop=(j == jmax))
                        nc.tensor.matmul(
                            otp[:, qlo:], vnat[:, j * 128:(j + 1) * 128],
                            pslice,
                            start=(j == 0), stop=(j == jmax))
                        if j == jmax:
                            # normalize: O^T * (1/sums), all [128, BW] DVE ops
                            rb_sb = rr_pool.tile([128, BW], f32, tag="rb")
                            nc.vector.reciprocal_approx_fast(rb_sb[:], smp[:])
                            nc.vector.tensor_mul(
                                ot_sb2[I % 2][:, h * BW:(h + 1) * BW],
                                otp[:], rb_sb[:])
                            del cur[(I, h)]
                            if h == HQ - 1:
                                pend += [(I, t, hf)
                                         for t in range(4) for hf in range(2)]
                                pend_from = idx + 2
                    for g in pend:
                        outproj_group(*g)

    nc.compile()
    return nc


def _get_nc():
    if "nc" not in _cache:
        _cache["nc"] = _build()
    return _cache["nc"]


def _host_inputs(x, Wq, Wk, Wv, Wo):
    import ml_dtypes
    bf = ml_dtypes.bfloat16
    x = np.asarray(x, dtype=np.float32)
    Wq = np.asarray(Wq, dtype=bf)
    Wk = np.asarray(Wk, dtype=bf)
    Wv = np.asarray(Wv, dtype=bf)
    Wo = np.asarray(Wo, dtype=bf)

    kk = np.arange(128)[:, None]
    qq = np.arange(128)[None, :]
    m01 = (qq >= kk).astype(bf)
    ident = np.eye(128, dtype=bf)
    onesd = np.ones((128, 128), dtype=bf)
    consts = np.ascontiguousarray(
        np.concatenate([m01, ident, onesd], axis=1))

    xTb = [np.ascontiguousarray(x[b].T).astype(bf) for b in range(B)]
    in_maps = []
    for c in range(8):
        b, g = c // 4, c % 4
        Wq_g = Wq[:, g * 512:(g + 1) * 512]
        xT = xTb[b]
        xw = np.empty((128, DC * CW), dtype=bf)
        wkv = np.empty((128, 2 * DC * HD), dtype=bf)
        for d in range(DC):
            xw[:, d * CW:d * CW + 512] = Wq_g[d * 128:(d + 1) * 128, :]
            xw[:, d * CW + 512:(d + 1) * CW] = xT[d * 128:(d + 1) * 128, :]
            wkv[:, d * HD:(d + 1) * HD] = \
                Wk[d * 128:(d + 1) * 128, g * HD:(g + 1) * HD]
            wkv[:, DC * HD + d * HD:DC * HD + (d + 1) * HD] = \
                Wv[d * 128:(d + 1) * 128, g * HD:(g + 1) * HD]
        in_maps.append({
            "xw": xw,
            "wkv": wkv,
            "wo": np.ascontiguousarray(Wo[g * 512:(g + 1) * 512, :]),
            "consts": consts,
        })
    return in_maps


def run(x, mask, Wq, Wk, Wv, Wo, trace=False, trace_cores=None):
    from concourse.bass_utils import run_bass_kernel_spmd

    nc = _get_nc()
    in_maps = _host_inputs(x, Wq, Wk, Wv, Wo)
    res = run_bass_kernel_spmd(
        nc, in_maps, core_ids=list(range(8)), trace=trace,
        trace_cores=trace_cores)
    full = np.empty((B, N, DIM), dtype=np.float32)
    for b in range(B):
        acc = res.results[b * 4 + 0]["out"].astype(np.float32)
        for g in range(1, 4):
            acc += res.results[b * 4 + g]["out"].astype(np.float32)
        full[b] = acc
    return full, res


def kernel(x, mask, Wq, Wk, Wv, Wo):
    out, _ = run(x, mask, Wq, Wk, Wv, Wo, trace=False)
    return out
